# revision 154
# baseline (speedup 1.0000x reference)
"""Multi-head attention Trainium2 Bass kernel, v8.

B=8, N=2048, C=768, H=12, D=64. Data-parallel over batch: 1 element/core.

Per-core pipeline:
  QKV q,k  : fp8e4 DoubleRow residual matmuls, hi*hi + hi*lo + lo*hi with
             UNSCALED lo residuals (fp8 is scale-free), so no /16 operand
             copies exist; 9 DR matmuls accumulate into ONE psum chain; the
             evacuation is a single-input downcast on ScalarE or DVE.
  V        : same residual-DR trick, key-major -> fp16 vaug [128, kt, 130]
             (two heads' 64+1 slots; ones col = softmax denominator).
             GPSIMD (otherwise idle) derives an fp8 RESIDUAL pair
             vaug8/vaug8l [128, kt, 160] for the DoubleRow AV path.
  S^T      : fp8 DoubleRow, D=64 split across the two DR slots (d = 2p+i on 32
             partitions); 0.5 cycles/row.  q8/k8 reach [32,2,N] via an
             SBUF->SBUF DMA fold (one DMA when a head-pair's slots align).
  exp      : two lanes, assigned per S-tile (GPSIMD/DMA have no PSUM port):
               ScalarE: true Exp -> fp8e4 (feeds DoubleRow AV)
               DVE:     Schraudolph magic in ONE tensor_scalar: uint16 out =
                        round(s*1024*log2(e)*scale + 15301.5), whose bit
                        pattern read as fp16 is ~exp(s*scale).
  AV       : per-qb 16-step chains into one packed psum bank [128, 4, 65]:
             ScalarE tiles run fp8 DR (2 matmuls/S-tile: V hi + lo residual),
             DVE tiles run fp16 (2 matmuls/S-tile) -- PE cost ~0.7x.
  norm     : DVE reciprocal + scalar_tensor_tensor broadcast-mult -> fp16 a2
             (DVE may read only one non-scalar PSUM input per op, so the
             denominator reciprocal goes through SBUF); aT via DMA xbar
             transpose (no PE, no psum).
  proj     : fp16 matmuls interleaved one chain per S-slot across head-pair
             5; ScalarE Identity (a few on DVE) applies 1/(SW*SP) + bias;
             yT [C, N] fp16 out (host transposes + upcasts).

Schedule: x streams in n-chunks interleaved with head-pair 0's k->q->v
chains so the first exp fires ~12us in; weight spans load lazily in head
order; all other input prep is pre-laid out on the host.
"""

import math

import numpy as np
import ml_dtypes

import concourse.bass as bass
import concourse.mybir as mybir
import concourse.tile as tile
from concourse import bacc
from concourse.bass_utils import run_bass_kernel_spmd

B, N, C, H = 8, 2048, 768, 12
D = C // H            # 64
CT = C // 128         # 6 channel tiles
NQ = 512              # query chunk (1 psum bank fp32)
NCH = N // NQ         # 4
NKT = N // 128        # 16 key tiles

SW = 32.0             # host scale on w_qkv
SP = 32.0             # host scale on w_proj
SCALE = float(D) ** -0.5
S_SCALE = SCALE / (SW * SW)        # exp scale on raw q8.k8 psum scores
LN2 = math.log(2.0)
A16 = 1024.0 * S_SCALE / LN2       # fp16 magic slope
B16 = 15301.5  # centered fp16 exponent bias (+.5 for trunc converts)

# Per-head-parity exp-lane assignment over the 8 S-tiles (pairs of ktiles).
# Act is the faster exp engine (0.83 vs 1.04 ns/row): give it the bigger share.
LANES = (
    "ADADADAA",   # head parity 0, chunk even
    "DADADAAD",   # head parity 1, chunk even
    "ADADADAA",   # head parity 0, chunk odd
    "DADADAAD",   # head parity 1, chunk odd
)
# evac lane for q/k f-major downcasts and v downcasts, by slice parity
QK_EVAC = ("A", "D")
V_EVAC = ("D", "A")

FP32 = mybir.dt.float32
FP16 = mybir.dt.float16
F8 = mybir.dt.float8e4
EXP = mybir.ActivationFunctionType.Exp
IDENT = mybir.ActivationFunctionType.Identity
COPY = mybir.ActivationFunctionType.Copy
MULT = mybir.AluOpType.mult
ADD = mybir.AluOpType.add
DIV = mybir.AluOpType.divide
DR = mybir.MatmulPerfMode.DoubleRow

F8NP = ml_dtypes.float8_e4m3

_CACHED_NC = None

# number of S-tiles (of 8 per (hp, ch, hi) block) sent to the ScalarE true-exp
# lane, per (hp, hi, ch) — tuned so ScalarE and DVE finish together, both
# globally and region-by-region (DVE is busier mid-kernel, ScalarE at the
# projection tail)
def _lanes(hp, hi, ch):
    if ch == 3 and hi == 1:
        return "DADADDAA" if hp != 0 else "DADADDAA"
    return LANES[hi + 2 * (ch % 2)]


def _ap(base, free_dims):
    """AP with base's partition dim and explicit [stride, count] free dims."""
    return bass.AP(
        tensor=base.tensor,
        offset=base.offset,
        ap=[list(base.ap[0])] + [list(d) for d in free_dims],
    )


def build():
    nc = bacc.Bacc()
    x_hi = nc.dram_tensor("x_hi", [128, CT, N], F8, kind="ExternalInput")
    x_lo = nc.dram_tensor("x_lo", [128, CT, N], F8, kind="ExternalInput")
    w_hi = nc.dram_tensor("w_hi", [128, CT, 3 * C], F8, kind="ExternalInput")
    w_lo = nc.dram_tensor("w_lo", [128, CT, 3 * C], F8, kind="ExternalInput")
    wp = nc.dram_tensor("wp", [128, CT, C], FP16, kind="ExternalInput")
    b2d = nc.dram_tensor("b2d", [128, CT], FP32, kind="ExternalInput")
    yT = nc.dram_tensor("yT", [C, N], FP16, kind="ExternalOutput")

    lp = nc.allow_low_precision("fp8/fp16 matmuls with fp32 psum accumulation")
    lp.__enter__()
    with tile.TileContext(nc) as tc:
        with tc.tile_pool(name="big", bufs=1) as big, \
             tc.tile_pool(name="fmp", bufs=2) as fmp, \
             tc.tile_pool(name="e16p", bufs=9) as e16p, \
             tc.tile_pool(name="e32p", bufs=8) as e32p, \
             tc.tile_pool(name="small", bufs=2) as small, \
             tc.tile_pool(name="ps2p", bufs=3, space="PSUM") as ps2p, \
             tc.tile_pool(name="psap", bufs=1, space="PSUM") as psap, \
             tc.tile_pool(name="psavp", bufs=1, space="PSUM") as psavp:

            # ---- persistent inputs -------------------------------------
            # loaded per ct-pair so the first DR chains start early; the lo
            # residuals are stored UNSCALED (fp8 is scale-free), so no /16
            # operand copies are needed
            xh = big.tile([128, CT, N], F8)
            xl = big.tile([128, CT, N], F8)
            wh = big.tile([128, CT, 3 * C], F8)
            wl = big.tile([128, CT, 3 * C], F8)
            # head-pair 0's weight f-slices first (tiny); x follows in
            # n-chunks interleaved with the hp-0 emission (below) so the tiny
            # fold DMAs outrank still-queued x chunks on the shared DMA engine
            for f0, f1 in ((768, 896), (0, 128), (1536, 1664)):
                nc.sync.dma_start(out=wh[:, :, f0:f1], in_=w_hi[:, :, f0:f1])
                nc.sync.dma_start(out=wl[:, :, f0:f1], in_=w_lo[:, :, f0:f1])

            def emit_x_chunk(n):
                nsl = slice(n * NQ, (n + 1) * NQ)
                nc.sync.dma_start(out=xh[:, :, nsl], in_=x_hi[:, :, nsl])
                nc.sync.dma_start(out=xl[:, :, nsl], in_=x_lo[:, :, nsl])
            wpt = big.tile([128, CT, C], FP16)
            bias = big.tile([128, CT], FP32)

            def emit_late_loads():
                # weight spans for head-pairs 1-5 and the proj weights: not
                # needed until the main loop is underway, so keep them out of
                # the DMA pipe ahead of the hp-0 q/k folds.  hp 1's slices
                # come first — its chains issue from the first S slot.
                for f0, f1 in ((128, 256), (896, 1024), (1664, 1792),
                               (256, 768), (1024, 1536), (1792, 2304)):
                    nc.sync.dma_start(out=wh[:, :, f0:f1], in_=w_hi[:, :, f0:f1])
                    nc.sync.dma_start(out=wl[:, :, f0:f1], in_=w_lo[:, :, f0:f1])
                nc.sync.dma_start(out=wpt, in_=wp[:, :, :])
                nc.sync.dma_start(out=bias, in_=b2d[:, :])

            # heads h = 2*hp + hi at group g = h // 3, slot q4 = h % 3
            # (matmul lhsT base partition must be 0/32/64, so 3 slots/group)
            qg = [big.tile([128, 2, N], F8, name=f"qg{g}") for g in range(4)]
            kg = [big.tile([128, 2, N], F8, name=f"kg{g}") for g in range(4)]
            vaug = [
                big.tile([128, NKT, 130], FP16, name=f"vaug{p}") for p in range(6)
            ]
            # fp8 residual copy of vaug for the DoubleRow AV path (A-lane
            # tiles): v ~= v8hi + v8lo elementwise, so the AV chain runs two
            # fp8-DR matmuls per S-tile instead of two fp16 ones.  The two
            # heads' 65-col halves sit at 0 and 80 so DR slices stay 16-byte
            # aligned.  GPSIMD (otherwise idle) produces both from fp16 vaug.
            vaug8 = [
                big.tile([128, NKT, 160], F8, name=f"vaug8_{p}") for p in range(6)
            ]
            vaug8l = [
                big.tile([128, NKT, 160], F8, name=f"vaug8l_{p}") for p in range(6)
            ]
            for p in range(6):
                nc.gpsimd.memset(vaug[p][:, :, 64:65], 1.0)
                nc.gpsimd.memset(vaug[p][:, :, 129:130], 1.0)
            aT = big.tile([128, CT, N], FP16)

            # pre-warm the ScalarE Exp table during the initial DMA wait so
            # LoadActFuncSet is off the first S tile's critical path
            warm = small.tile([128, 2], FP32, tag="warm", bufs=1, name="warm")
            nc.gpsimd.memset(warm, 0.0)
            nc.scalar.activation(out=warm[:, 0:1], in_=warm[:, 1:2],
                                 func=EXP, scale=1.0)

            def emit_vaug8(hp):
                v16 = vaug[hp]
                v8 = vaug8[hp]
                v8l = vaug8l[hp]
                src = _ap(v16[:, 0, 0:65], [[130, NKT], [65, 2], [1, 65]])
                dst = _ap(v8[:, 0, 0:65], [[160, NKT], [80, 2], [1, 65]])
                dstl = _ap(v8l[:, 0, 0:65], [[160, NKT], [80, 2], [1, 65]])
                nc.gpsimd.tensor_copy(dst, src)
                # residual: v8l = v16 - v8 (rounded to fp8)
                nc.gpsimd.tensor_tensor(
                    out=dstl, in0=src,
                    in1=dst,
                    op=mybir.AluOpType.subtract,
                )

            # residual-DR chain: 9 matmuls into one psum, full precision:
            # hi*hi + hi*lo + lo*hi (unscaled lo).  ct-pair outermost so the
            # chain starts as soon as the first input chunks land.
            def dr_chain(ps_out, pairs):
                k = 0
                for t in range(3):
                    for (lt, lsl), (rt, rsl) in pairs:
                        nc.tensor.matmul(
                            ps_out,
                            lt[:, 2 * t:2 * t + 2, lsl],
                            rt[:, 2 * t:2 * t + 2, rsl],
                            start=(k == 0), stop=(k == 8), perf_mode=DR,
                        )
                        k += 1

            def evac(lane, out, in_):
                if lane == "A":
                    nc.scalar.activation(out=out, in_=in_, func=COPY, scale=1.0)
                else:
                    nc.vector.tensor_copy(out, in_)

            # ---- phase A pieces ----------------------------------------
            def emit_qk_side(hp, ch, side, fms):
                n0 = ch * NQ
                nsl = slice(n0, n0 + NQ)
                # hp 0 runs before any S work: borrow the (idle) S-psum
                # rotation so its chains pipeline instead of serializing
                # through the single psa bank
                if hp == 0:
                    ps = ps2p.tile([128, NQ], FP32, tag="ps2", name="psqk0")
                else:
                    ps = psap.tile([128, NQ], FP32, tag="psa", name="psqk")
                f0 = 128 * (hp + 6 * side)
                fsl = slice(f0, f0 + 128)
                dr_chain(
                    ps[:, :],
                    (((wh, fsl), (xh, nsl)),
                     ((wh, fsl), (xl, nsl)),
                     ((wl, fsl), (xh, nsl))),
                )
                evac(QK_EVAC[(ch + side) % 2],
                     fms[side][:, n0:n0 + NQ], ps[:, :])

            def emit_v_half(hp, ch, j):
                # 2 key-tiles of this pair's V: kts {4ch+2j, 4ch+2j+1}
                vf0 = 1536 + 128 * hp
                vsl = slice(vf0, vf0 + 128)
                if hp == 0:
                    ps = ps2p.tile([128, NQ], FP32, tag="ps2", name="psv0")
                else:
                    ps = psap.tile([128, NQ], FP32, tag="psa", name="psv")
                for i in (0, 1):
                    kt = 4 * ch + 2 * j + i
                    n0 = kt * 128
                    nsl = slice(n0, n0 + 128)
                    dr_chain(
                        ps[:, 128 * i:128 * i + 128],
                        (((xh, nsl), (wh, vsl)),
                         ((xl, nsl), (wh, vsl)),
                         ((xh, nsl), (wl, vsl))),
                    )
                vrow = vaug[hp][:, 4 * ch + 2 * j, :]
                out = _ap(vrow, [[130, 2], [65, 2], [1, 64]])
                src = _ap(ps[:, :], [[128, 2], [64, 2], [1, 64]])
                evac(V_EVAC[(ch + j) % 2], out, src)

            def new_fm():
                fq = fmp.tile([128, N], F8, tag="fmq", name="fq")
                fk = fmp.tile([128, N], F8, tag="fmk", name="fk")
                return fq, fk

            def emit_a_slice(hp, ch, fms):
                emit_qk_side(hp, ch, 0, fms)
                emit_v_half(hp, ch, 0)
                emit_qk_side(hp, ch, 1, fms)
                emit_v_half(hp, ch, 1)

            def emit_rearrange_ch(hp, ch, fms, sides=None):
                # fold the [128, NQ] f-major chunk into [32, 2, NQ] slots
                # (d = 2p + i); when the pair's two heads land in adjacent
                # slots of one group, fold both with a single DMA.  Kept
                # per-chunk: a fold with unmet waits blocks the whole SP
                # DMA queue behind it.
                n0 = ch * NQ
                h0 = 2 * hp
                g0, q40 = h0 // 3, h0 % 3
                g1, q41 = (h0 + 1) // 3, (h0 + 1) % 3
                for side, grps in sides if sides else ((0, qg), (1, kg)):
                    if g0 == g1:
                        nc.sync.dma_start(
                            out=grps[g0][32 * q40:32 * q40 + 64, :, n0:n0 + NQ],
                            in_=fms[side][:, n0:n0 + NQ],
                        )
                    else:
                        for hi, g, q4 in ((0, g0, q40), (1, g1, q41)):
                            nc.sync.dma_start(
                                out=grps[g][32 * q4:32 * q4 + 32, :, n0:n0 + NQ],
                                in_=fms[side][64 * hi:64 * hi + 64, n0:n0 + NQ],
                            )

            # ---- phase B: S+exp for head k runs while head k-1's AV/norm
            # retires, so the exp engines always have fresh psums ------------
            def emit_s_exp(hp, ch, hi, sts=None, ets=None):
                n0 = ch * NQ
                h = 2 * hp + hi
                g, q4 = h // 3, h % 3
                p0 = 32 * q4
                qs = qg[g][p0:p0 + 32, :, n0:n0 + NQ]
                lanes = _lanes(hp, hi, ch)
                if sts is None:
                    sts = range(8)
                if ets is None:
                    ets = []
                for st in sts:
                    ps = ps2p.tile([128, 2, NQ], FP32, tag="ps2", name="sps")
                    for tt in (0, 1):
                        kt = 2 * st + tt
                        nc.tensor.matmul(
                            ps[:, tt, :],
                            kg[g][p0:p0 + 32, :, kt * 128:kt * 128 + 128],
                            qs,
                            start=True, stop=True, perf_mode=DR,
                        )
                    if lanes[st] == "A":
                        # true exp straight to fp8: feeds the DoubleRow AV
                        et = e16p.tile([128, 2, NQ], F8, tag="e16", name="e8")
                        nc.scalar.activation(
                            out=et, in_=ps, func=EXP, scale=S_SCALE
                        )
                        ets.append((et, False))
                    else:
                        # fp32 -> uint16 convert IS the magic: i16 lands as the
                        # fp16 bit pattern of ~exp(s)
                        et = e32p.tile([128, 2, NQ], mybir.dt.uint16,
                                       tag="e32", name="e32")
                        nc.vector.tensor_scalar(
                            out=et, in0=ps, scalar1=A16, scalar2=B16,
                            op0=MULT, op1=ADD,
                        )
                        ets.append((et, True))
                return (hp, ch, hi, ets)

            a2map = {}

            def emit_av_norm(ctx):
                hp, ch, hi, ets = ctx
                n0 = ch * NQ
                if hi == 0:
                    a2map[(hp, ch)] = small.tile(
                        [128, 4, 128], FP16, tag="a2", name="a2", bufs=2
                    )
                a2 = a2map[(hp, ch)]
                av = psavp.tile([128, 4, 128], FP32, tag="av", name="av")
                for qb in range(4):
                    for st in range(8):
                        et, magic = ets[st]
                        if magic:
                            # fp16 path: one matmul per key tile
                            for tt in (0, 1):
                                kt = 2 * st + tt
                                nc.tensor.matmul(
                                    av[:, qb, 0:65],
                                    et.bitcast(FP16)[:, tt, qb * 128:qb * 128 + 128],
                                    vaug[hp][:, kt, 65 * hi:65 * hi + 65],
                                    start=(st == 0 and tt == 0),
                                    stop=(st == 7 and tt == 1),
                                )
                        else:
                            # fp8 DoubleRow: both key tiles per matmul; two
                            # matmuls apply the hi + lo residual of V
                            for vsrc in (vaug8, vaug8l):
                                nc.tensor.matmul(
                                    av[:, qb, 0:65],
                                    et[:, :, qb * 128:qb * 128 + 128],
                                    vsrc[hp][:, 2 * st:2 * st + 2,
                                             80 * hi:80 * hi + 65],
                                    start=(st == 0 and vsrc is vaug8),
                                    stop=(st == 7 and vsrc is vaug8l),
                                    perf_mode=DR,
                                )
                # DVE may read only ONE non-scalar PSUM input per op, so the
                # denominator goes through a tiny SBUF reciprocal first
                recp = small.tile([128, 4], FP32, tag="recp", name="recp")
                nc.vector.reciprocal(recp, av[:, :, 64])
                nc.vector.scalar_tensor_tensor(
                    out=a2[:, :, 64 * hi:64 * hi + 64],
                    in0=av[:, :, 0:64],
                    scalar=1.0,
                    in1=_ap(recp, [[1, 4], [0, 64]]),
                    op0=MULT, op1=MULT,
                )
                if hi == 1:
                    nc.sync.dma_start_transpose(
                        out=aT[:, hp, n0:n0 + NQ].rearrange(
                            "p (qb q) -> p qb q", qb=4
                        ),
                        in_=a2.rearrange("p qb d -> p (qb d)"),
                    )
                    del a2map[(hp, ch)]
                    if hp == 5:
                        # all 6 head-pairs of this chunk are in aT: queue its
                        # projection chains; they drain one per S/exp slot so
                        # they overlap the remaining attention work
                        proj_pend.extend((ch, ot) for ot in range(6))

            # ---- phase C (one (ch, ot) chain at a time) -----------------
            proj_pend = []
            proj_ct = [0]

            def drain_proj(k=1, mid=False):
                for _ in range(min(k, len(proj_pend))):
                    ch, ot = proj_pend.pop(0)
                    n0 = ch * NQ
                    if mid:
                        # while S/exp still runs, keep proj out of the ps2
                        # rotation (an S psum stalling behind a proj chain
                        # starves the exp engines); psa is idle by hp 5
                        psl = psap.tile([128, NQ], FP32, tag="psa",
                                        name="cpsa")
                    else:
                        ps = ps2p.tile([128, 2, NQ], FP32, tag="ps2",
                                       name="cps")
                        psl = ps[:, 0, :]
                    for ct in range(CT):
                        nc.tensor.matmul(
                            psl,
                            wpt[:, ct, 128 * ot:128 * ot + 128],
                            aT[:, ct, n0:n0 + NQ],
                            start=(ct == 0), stop=(ct == CT - 1),
                        )
                    ysb = e32p.tile([128, NQ], FP16, tag="ysb", bufs=2,
                                    name="ysb")
                    # last few evacs on DVE — it idles once the exp work dries
                    # up at the projection tail
                    if proj_ct[0] < 19:
                        nc.scalar.activation(
                            out=ysb, in_=psl, func=IDENT,
                            scale=1.0 / (SW * SP), bias=bias[:, ot:ot + 1],
                        )
                    else:
                        nc.vector.scalar_tensor_tensor(
                            out=ysb, in0=psl,
                            scalar=1.0 / (SW * SP),
                            in1=_ap(bias[:, ot:ot + 1], [[0, NQ]]),
                            op0=MULT, op1=ADD,
                        )
                    proj_ct[0] += 1
                    nc.sync.dma_start(
                        out=yT[128 * ot:128 * ot + 128, n0:n0 + NQ], in_=ysb
                    )

            # ---- emission: A slices pipelined one pair ahead of B, AV
            # blocks deferred one head behind their S+exp ------------------
            # hp 0 startup: k side lands progressively; the first S/exp block
            # (0,0,0) is emitted two key-tiles at a time right behind each k
            # chunk's fold so exp work starts as soon as x is on chip
            fms = new_fm()
            for ch in range(NCH):
                emit_x_chunk(ch)
                emit_qk_side(0, ch, 1, fms)
                emit_rearrange_ch(0, ch, fms, sides=((1, kg),))
                emit_qk_side(0, ch, 0, fms)
                emit_rearrange_ch(0, ch, fms, sides=((0, qg),))
                emit_v_half(0, ch, 0)
                emit_v_half(0, ch, 1)
            emit_vaug8(0)
            emit_late_loads()
            pend = None
            for hp in range(6):
                nfms = new_fm() if hp < 5 else None
                for ch in range(NCH):
                    for hi in (0, 1):
                        ctx = emit_s_exp(hp, ch, hi)
                        if pend is not None:
                            emit_av_norm(pend)
                        pend = ctx
                        if hp < 5:
                            if hi == 0:
                                # both sides in the first slot (psa absorbs
                                # two chain+evac cycles per slot) so the
                                # folds gating the next hp land a slot early
                                emit_qk_side(hp + 1, ch, 0, nfms)
                                emit_qk_side(hp + 1, ch, 1, nfms)
                                emit_rearrange_ch(hp + 1, ch, nfms)
                            emit_v_half(hp + 1, ch, hi)
                            if hi == 1 and ch == 3:
                                emit_vaug8(hp + 1)
                        else:
                            drain_proj(2, mid=True)
                            if (ch == 2 and hi == 1) or ch == 3:
                                # exp is nearly done: an extra chain through
                                # the ps2 rotation no longer starves it
                                drain_proj(1)
            emit_av_norm(pend)
            drain_proj(24)
    lp.__exit__(None, None, None)

    nc.finalize()
    return nc


def get_nc():
    global _CACHED_NC
    if _CACHED_NC is None:
        _CACHED_NC = build()
    return _CACHED_NC


def _prep_shared(w_qkv, w_proj, b_proj):
    wq = (w_qkv.astype(np.float64) * SW).astype(np.float32)
    w_hi = wq.astype(F8NP)
    w_lo = (wq - w_hi.astype(np.float32)).astype(F8NP)  # unscaled residual

    def lay_w(a):
        return np.ascontiguousarray(a.T.reshape(CT, 128, 3 * C).transpose(1, 0, 2))
    wpm = (w_proj.astype(np.float64) * SP).astype(np.float16)
    wp_l = np.ascontiguousarray(wpm.T.reshape(CT, 128, C).transpose(1, 0, 2))
    b2d = np.ascontiguousarray(b_proj.reshape(CT, 128).T.astype(np.float32))
    return lay_w(w_hi), lay_w(w_lo), wp_l, b2d


def _prep_x(xi):
    xs = np.ascontiguousarray(xi.T.reshape(CT, 128, N).transpose(1, 0, 2))
    x_hi = xs.astype(F8NP)
    x_lo = (xs - x_hi.astype(np.float32)).astype(F8NP)  # unscaled residual
    return x_hi, x_lo


LAST_RESULT = None


def kernel(x, w_qkv, w_proj, b_proj, **run_kwargs):
    x = np.ascontiguousarray(np.asarray(x, dtype=np.float32))
    w_qkv = np.ascontiguousarray(np.asarray(w_qkv, dtype=np.float32))
    w_proj = np.ascontiguousarray(np.asarray(w_proj, dtype=np.float32))
    b_proj = np.ascontiguousarray(np.asarray(b_proj, dtype=np.float32))
    assert x.shape == (B, N, C)

    nc = get_nc()
    w_hi, w_lo_l, wp_l, b2d = _prep_shared(w_qkv, w_proj, b_proj)
    in_maps = []
    for i in range(B):
        x_hi, x_lo = _prep_x(x[i])
        in_maps.append({
            "x_hi": x_hi, "x_lo": x_lo,
            "w_hi": w_hi, "w_lo": w_lo_l,
            "wp": wp_l, "b2d": b2d,
        })
    res = run_bass_kernel_spmd(nc, in_maps, list(range(B)), **run_kwargs)
    global LAST_RESULT
    LAST_RESULT = res
    out = np.stack(
        [np.ascontiguousarray(res.results[i]["yT"].T.astype(np.float32))
         for i in range(B)], axis=0
    )
    return out


if __name__ == "__main__":
    rng = np.random.default_rng(0)
    x = rng.standard_normal((B, N, C), dtype=np.float32)
    w_qkv = (rng.standard_normal((3 * C, C)) * 0.02).astype(np.float32)
    w_proj = (rng.standard_normal((C, C)) * 0.02).astype(np.float32)
    b_proj = (rng.standard_normal((C,)) * 0.02).astype(np.float32)
    out = kernel(x=x, w_qkv=w_qkv, w_proj=w_proj, b_proj=b_proj)
    print("out", out.shape, out.dtype, float(np.abs(out).max()))

